# revision 23
# baseline (speedup 1.0000x reference)
"""Multi-head causal self-attention (B=2, T=2048, D=1024, H=16) on 8 trn2 cores.

Sharding: core c handles batch b=c//4 and head-group g=c%4 (4 heads, 256 feats).
The kernel streams token supertiles of 512: for each ts it computes Q/K/V for
the core's 4 heads, runs causal attention for query block qs=ts (K/V for all
needed kv blocks are already resident -- causal), multiplies the local 256
attention features into a partial output [1024, 512] against w_o rows, and
ReduceScatters that partial (bf16, op=add) across the 4 cores of the batch
group -- each core receives its own 256 output features for those 512 tokens.
The 4 chunked ReduceScatters overlap the next supertile's compute, replacing
the two serialized full-width AllGathers of the tensor-parallel formulation.

All matmuls run in fp32r (full-rate PE). Softmax skips the running max
(scores bounded ~N(0,1) by construction); the 1/sqrt(Dh) scale is folded into
the Exp activation; the denominator comes from a ones-column appended to V.
Scores are computed as S^T[k, q] so softmax reduces along the free axis and
P@V consumes exp(S^T) directly -- no transposes. The two heads of an f-tile
sit at partition bases 0/64 so their K=64 score matmuls occupy disjoint PE
row-groups; their exps fuse into one 1024-column ACT op. S of block kb+1 is
issued before PV of block kb so the PE never waits on the Exp. Warm-up
matmuls on the resident mask tile ramp the PE clock under the input-DMA
lead-in. b_o/4 is added to each partial pre-reduce, so the RS output is final.
"""

import os
import sys

for _p in ("/opt/trn_rl_repo", "/root/.axon_site/_ro/trn_rl_repo"):
    if os.path.isdir(_p) and _p not in sys.path:
        sys.path.insert(0, _p)

import numpy as np

import concourse.bacc as bacc
import concourse.mybir as mybir
import concourse.tile as tile
from concourse.bass_utils import run_bass_kernel_spmd

F32 = mybir.dt.float32
F32R = mybir.dt.float32r
BF16 = mybir.dt.bfloat16
AF = mybir.ActivationFunctionType

B, T, C = 2, 2048, 1024
H, Dh = 16, 64
NCORES, GRP = 8, 4        # 2 groups of 4 cores (one per batch)
HL, FL = 4, 256           # heads / features per core
TS = 512                  # token supertile
NQ = T // TS              # 4
JL = C // GRP             # 256 output features per core after the RS

_CACHE = {}
_TRACE = False
_LAST = None


def _build(unroll=1):
    nc = bacc.Bacc("TRN2", target_bir_lowering=False, debug=False,
                   num_devices=NCORES)

    xT = nc.dram_tensor("xT", [C, T], F32R, kind="ExternalInput")
    wqT = nc.dram_tensor("wqT", [C, FL], F32R, kind="ExternalInput")
    wkT = nc.dram_tensor("wkT", [C, FL], F32R, kind="ExternalInput")
    wvT = nc.dram_tensor("wvT", [C, FL], F32R, kind="ExternalInput")
    woL = nc.dram_tensor("woL", [FL, C], F32R, kind="ExternalInput")
    bqk_row = nc.dram_tensor("bqk_row", [1, 4, 128], F32R, kind="ExternalInput")
    ones_in = nc.dram_tensor("ones_in", [128, 64], BF16, kind="ExternalInput")
    ones_rin = nc.dram_tensor("ones_rin", [1, TS], F32R, kind="ExternalInput")
    bv_row = nc.dram_tensor("bv_row", [1, FL], F32R, kind="ExternalInput")
    bo_bc = nc.dram_tensor("bo_bc", [128, 8], F32, kind="ExternalInput")
    mask2 = nc.dram_tensor("mask2", [128, 2, 128], BF16, kind="ExternalInput")
    outRS = nc.dram_tensor("outRS", [NQ, JL, TS], BF16, kind="ExternalOutput")

    with tile.TileContext(nc) as tc:
        for _it in range(unroll):
            with tc.tile_pool(name="persist", bufs=1) as pp:
                # ---- persistent SBUF state ----
                QT = pp.tile([128, 2, T], F32R)          # Q^T  [f, t]
                KT = pp.tile([128, 2, T], F32R)          # K^T  [f, t]
                Vg = pp.tile([128, T // 128, HL, Dh + 1], BF16)
                attnT = pp.tile([128, 2, T], F32R)       # attention out^T
                mask_sb = pp.tile([128, 2, 128], BF16)
                bqkr_sb = pp.tile([1, 4, 128], F32R)
                bvr_sb = pp.tile([1, FL], F32R)
                bo_sb = pp.tile([128, 8], F32)
                ones_sb = pp.tile([128, 64], BF16)
                ones_row = pp.tile([1, TS], F32R)

                nc.sync.dma_start(mask_sb[:], mask2[:])
                # PE warm-up ramps the HAM clock under the input-DMA lead-in;
                # psum is discarded.
                with tc.tile_pool(name="warm", bufs=1, space="PSUM") as wp:
                    ps_w = wp.tile([128, 256], F32, name="ps_w")
                    for _w in range(50):
                        nc.tensor.matmul(
                            ps_w[:], lhsT=mask_sb[:, 0, :],
                            rhs=mask_sb.rearrange("p a b -> p (a b)"),
                            start=True, stop=True)

                dp = tc.tile_pool(name="dram", bufs=1, space="DRAM")
                dpp = dp.__enter__()
                rs_in = dpp.tile([NQ, C, TS], BF16)
                rs_out = dpp.tile([NQ, JL, TS], BF16)

                with tc.tile_pool(name="xw", bufs=1) as xw, \
                     tc.tile_pool(name="att", bufs=3) as att, \
                     tc.tile_pool(name="fin2", bufs=2) as fin2, \
                     tc.tile_pool(name="psA", bufs=2, space="PSUM") as psA, \
                     tc.tile_pool(name="psS", bufs=2, space="PSUM") as psS, \
                     tc.tile_pool(name="psO", bufs=2, space="PSUM") as psO:
                    xT_sb = xw.tile([128, 8, T], F32R)
                    wq_sb = xw.tile([128, 8, FL], F32R)
                    wk_sb = xw.tile([128, 8, FL], F32R)
                    wv_sb = xw.tile([128, 8, FL], F32R)
                    wo_sb = xw.tile([128, 2, C], F32R)
                    # coalesced loads, one strided DMA each (HWDGE is paced
                    # at ~625 ns per DMACopy), ordered by first use
                    xTr = xT.rearrange("(c p) t -> p c t", p=128)
                    nc.sync.dma_start(xT_sb[:, 0:4, 0:TS], xTr[:, 0:4, 0:TS])
                    nc.sync.dma_start(xT_sb[:, 4:8, 0:TS], xTr[:, 4:8, 0:TS])
                    nc.sync.dma_start(
                        wq_sb[:], wqT.rearrange("(c p) f -> p c f", p=128))
                    nc.sync.dma_start(bqkr_sb[:], bqk_row[:])
                    nc.sync.dma_start(ones_row[:], ones_rin[:])
                    nc.sync.dma_start(bvr_sb[:], bv_row[:])
                    nc.sync.dma_start(ones_sb[:], ones_in[:])
                    # softmax denominator column of V via a strided ACT copy
                    nc.scalar.activation(
                        Vg[:, :, :, Dh:Dh + 1],
                        ones_sb.rearrange("p (a b o) -> p a b o", a=T // 128,
                                          b=HL), AF.Copy)
                    nc.sync.dma_start(
                        wk_sb[:], wkT.rearrange("(c p) f -> p c f", p=128))
                    nc.sync.dma_start(
                        wv_sb[:], wvT.rearrange("(c p) f -> p c f", p=128))
                    nc.sync.dma_start(xT_sb[:, :, TS:2 * TS], xTr[:, :, TS:2 * TS])
                    nc.sync.dma_start(
                        wo_sb[:], woL.rearrange("(c p) j -> p c j", p=128))
                    nc.sync.dma_start(bo_sb[:], bo_bc[:])
                    for ts_ in range(2, NQ):
                        nc.sync.dma_start(
                            xT_sb[:, :, ts_ * TS:(ts_ + 1) * TS],
                            xTr[:, :, ts_ * TS:(ts_ + 1) * TS])

                    def qkv_gen(ts_):
                        # Q^T,K^T [f, t] for both f-tiles of this supertile.
                        # Biases ride as an extra rank-1 accumulate matmul so
                        # the psum drain is a pure ACT copy -- DVE stays free
                        # for the softmax-normalize chain.
                        for dst, w_sb, bcol in ((QT, wq_sb, 0), (KT, wk_sb, 2)):
                            for ft in range(2):
                                ps = psA.tile([128, TS], F32, name="ps_qk",
                                              tag="psA")
                                nc.tensor.matmul(
                                    ps[:], lhsT=bqkr_sb[0:1, bcol + ft, :],
                                    rhs=ones_row[:], start=True, stop=False)
                                for cc in range(8):
                                    nc.tensor.matmul(
                                        ps[:],
                                        lhsT=w_sb[:, cc, ft * 128:(ft + 1) * 128],
                                        rhs=xT_sb[:, cc, ts_ * TS:(ts_ + 1) * TS],
                                        start=False, stop=(cc == 7))
                                nc.scalar.activation(
                                    dst[:, ft, ts_ * TS:(ts_ + 1) * TS], ps[:],
                                    AF.Copy)
                                yield
                        # V token-major: [t, f] = sum_c x^T[c, t] w_v^T[c, f]
                        for tb in range(4 * ts_, 4 * ts_ + 4):
                            ps = psA.tile([128, TS], F32, name="ps_v",
                                          tag="psA")[:, :FL]
                            nc.tensor.matmul(
                                ps[:], lhsT=ones_row[0:1, 0:128],
                                rhs=bvr_sb[:], start=True, stop=False)
                            for cc in range(8):
                                nc.tensor.matmul(
                                    ps[:],
                                    lhsT=xT_sb[:, cc, tb * 128:(tb + 1) * 128],
                                    rhs=wv_sb[:, cc, :],
                                    start=False, stop=(cc == 7))
                            nc.scalar.activation(
                                Vg[:, tb, :, 0:Dh],
                                ps.rearrange("p (h d) -> p h d", h=HL),
                                AF.Copy)
                            yield

                    def att_gen(qs):
                        # all 4 heads for query supertile qs; heads (2ft, 2ft+1)
                        # at partition bases (0, 64)
                        for ft in range(2):
                            Q0, K0 = QT[0:64, ft, :], KT[0:64, ft, :]
                            Q1, K1 = QT[64:128, ft, :], KT[64:128, ft, :]
                            h0, h1 = 2 * ft, 2 * ft + 1
                            po0 = psO.tile([128, TS], F32, name="po0",
                                           tag="ps_o")
                            po1 = psO.tile([128, TS], F32, name="po1",
                                           tag="ps_o")
                            nkb = 4 * qs + 4

                            def s_part(kb, q_lo):
                                # S^T for both heads at kv block kb -> exp
                                ps_s = psS.tile([128, 2, TS], F32,
                                                name="ps_s", tag="ps_s")
                                nc.tensor.matmul(
                                    ps_s[:, 0, q_lo:TS],
                                    lhsT=K0[:, kb * 128:(kb + 1) * 128],
                                    rhs=Q0[:, qs * TS + q_lo:(qs + 1) * TS],
                                    start=True, stop=True)
                                nc.tensor.matmul(
                                    ps_s[:, 1, q_lo:TS],
                                    lhsT=K1[:, kb * 128:(kb + 1) * 128],
                                    rhs=Q1[:, qs * TS + q_lo:(qs + 1) * TS],
                                    start=True, stop=True)
                                p_sb = att.tile([128, 2, TS], BF16,
                                                name="p_sb", tag="p")
                                nc.scalar.activation(
                                    p_sb[:, :, q_lo:TS], ps_s[:, :, q_lo:TS],
                                    AF.Exp, scale=0.125)
                                diag = kb - 4 * qs
                                if diag >= 0:  # triangular mask
                                    mo = diag * 128
                                    nc.vector.tensor_mul(
                                        p_sb[:, :, mo:mo + 128],
                                        p_sb[:, :, mo:mo + 128],
                                        mask_sb[:])
                                return p_sb

                            def pv_part(kb, q_lo, p_sb):
                                nc.tensor.matmul(
                                    po0[0:65, q_lo:TS],
                                    lhsT=Vg[:, kb, h0, :],
                                    rhs=p_sb[:, 0, q_lo:TS],
                                    start=(kb == 0), stop=(kb == nkb - 1))
                                nc.tensor.matmul(
                                    po1[0:65, q_lo:TS],
                                    lhsT=Vg[:, kb, h1, :],
                                    rhs=p_sb[:, 1, q_lo:TS],
                                    start=(kb == 0), stop=(kb == nkb - 1))

                            # one-block software pipeline: S(kb+1) issues
                            # before PV(kb) so PE never waits on the Exp
                            qlo = lambda kb: max(0, (kb - 4 * qs) * 128)
                            prev = None
                            for kb in range(nkb):
                                cur = (kb, qlo(kb), s_part(kb, qlo(kb)))
                                if prev is not None:
                                    pv_part(*prev)
                                prev = cur
                                yield
                            pv_part(*prev)

                            # normalize both heads by the ones-column sums
                            for po, fb in ((po0, 0), (po1, 64)):
                                r_sb = att.tile([1, TS], F32R, name="r_sb",
                                                tag="r")
                                with nc.allow_low_precision(reason="f32r"):
                                    nc.vector.reciprocal(r_sb[:],
                                                         po[64:65, :])
                                r_bc = att.tile([64, TS], F32R, name="r_bc",
                                                tag="r_bc", bufs=2)
                                nc.gpsimd.partition_broadcast(
                                    r_bc[:], r_sb[:])
                                nc.vector.tensor_mul(
                                    attnT[fb:fb + 64, ft,
                                          qs * TS:(qs + 1) * TS],
                                    po[0:64, :], r_bc[:])
                            yield

                    def wo_gen(ts_, deep=False):
                        # partial out^T[j, t] = w_o[j, f_local] attnT[f_local, t]
                        # (+ b_o/4) -> bf16 -> RS input; epilogues alternate
                        # ACT/DVE so neither engine paces the psum recycling
                        o_sb = fin2.tile([128, 8, TS], BF16, name="o_sb",
                                         tag="o")
                        for jt in range(8):
                            if deep and jt % 2:
                                ps = psO.tile([128, TS], F32, name="po0",
                                              tag="ps_o")
                            else:
                                ps = psA.tile([128, TS], F32, name="ps_f",
                                              tag="psA")
                            for fc in range(2):
                                nc.tensor.matmul(
                                    ps[:],
                                    lhsT=wo_sb[:, fc, jt * 128:(jt + 1) * 128],
                                    rhs=attnT[:, fc, ts_ * TS:(ts_ + 1) * TS],
                                    start=(fc == 0), stop=(fc == 1))
                            # half-width epilogues on ACT and DVE in parallel
                            # keep the psum recycle faster than the matmuls
                            with nc.allow_low_precision(reason="bf16 partial"):
                                nc.scalar.activation(
                                    o_sb[:, jt, 0:TS // 2], ps[:, 0:TS // 2],
                                    AF.Identity, bias=bo_sb[:, jt:jt + 1])
                                nc.vector.tensor_scalar_add(
                                    o_sb[:, jt, TS // 2:TS],
                                    ps[:, TS // 2:TS], bo_sb[:, jt:jt + 1])
                            yield
                        rsv = rs_in[ts_].rearrange("(j p) t -> p j t", p=128)
                        nc.sync.dma_start(rsv[:, 0:4, :], o_sb[:, 0:4, :])
                        nc.sync.dma_start(rsv[:, 4:8, :], o_sb[:, 4:8, :])
                        nc.gpsimd.collective_compute(
                            "ReduceScatter", mybir.AluOpType.add,
                            replica_groups=[[0, 1, 2, 3], [4, 5, 6, 7]],
                            ins=[rs_in[ts_].opt()], outs=[rs_out[ts_].opt()])

                    def drain(g):
                        for _ in g:
                            pass

                    def weave(main, n_main, fillers, n_fill):
                        # spread n_fill filler steps evenly across the n_main
                        # steps of the ACT-bound attention so its exp chain
                        # hides under the fillers' PE-only matmul groups
                        fi = 0
                        credit = 0.0
                        for _ in main:
                            credit += n_fill / n_main
                            while credit >= 1.0 and fi < len(fillers):
                                try:
                                    next(fillers[fi])
                                    credit -= 1.0
                                except StopIteration:
                                    fi += 1
                        for g in fillers[fi:]:
                            drain(g)

                    drain(qkv_gen(0))
                    weave(att_gen(0), 10, [qkv_gen(1)], 12)
                    weave(att_gen(1), 18, [wo_gen(0), qkv_gen(2)], 20)
                    weave(att_gen(2), 26, [wo_gen(1), qkv_gen(3)], 20)
                    weave(att_gen(3), 34, [wo_gen(2)], 8)
                    drain(wo_gen(3, deep=True))
                    # output copies last: a copy's wait on its RS would block
                    # SP.SEQ and delay later rs_in stages if emitted inline
                    for ts_ in range(NQ):
                        nc.sync.dma_start(outRS[ts_], rs_out[ts_])
                dp.__exit__(None, None, None)

    nc.compile()
    return nc


def _bf(a):
    import ml_dtypes
    return np.asarray(a, dtype=ml_dtypes.bfloat16)


def _make_in_maps(x, w_q, b_q, w_k, b_k, w_v, b_v, w_o, b_o):
    x = np.asarray(x, dtype=np.float32)
    w_q = np.asarray(w_q, dtype=np.float32)
    w_k = np.asarray(w_k, dtype=np.float32)
    w_v = np.asarray(w_v, dtype=np.float32)
    w_o = np.asarray(w_o, dtype=np.float32)
    b_q = np.asarray(b_q, dtype=np.float32)
    b_k = np.asarray(b_k, dtype=np.float32)
    b_v = np.asarray(b_v, dtype=np.float32)
    b_o = np.asarray(b_o, dtype=np.float32)

    mask_t = np.triu(np.ones((128, 128), dtype=np.float32))
    xTs = [np.ascontiguousarray(x[b].T) for b in range(B)]
    bo_t = np.ascontiguousarray((b_o / GRP).reshape(8, 128).T)

    in_maps = []
    for c in range(NCORES):
        b, g = c // GRP, c % GRP
        fsl = slice(g * FL, (g + 1) * FL)
        bqk_r = np.concatenate([b_q[fsl].reshape(2, 128),
                                b_k[fsl].reshape(2, 128)])[None]  # [1, 4, 128]
        in_maps.append({
            "xT": xTs[b],
            "wqT": np.ascontiguousarray(w_q[fsl, :].T),
            "wkT": np.ascontiguousarray(w_k[fsl, :].T),
            "wvT": np.ascontiguousarray(w_v[fsl, :].T),
            "woL": np.ascontiguousarray(w_o[:, fsl].T),
            "bqk_row": np.ascontiguousarray(bqk_r),
            "bv_row": np.ascontiguousarray(b_v[fsl][None]),
            "bo_bc": bo_t,
            "mask2": _bf(np.ascontiguousarray(
                np.repeat(mask_t[:, None, :], 2, axis=1))),
            "ones_in": _bf(np.ones((128, 64), dtype=np.float32)),
            "ones_rin": np.ones((1, TS), dtype=np.float32),
        })
    return in_maps


def kernel(x, w_q, b_q, w_k, b_k, w_v, b_v, w_o, b_o):
    global _LAST
    if "nc" not in _CACHE:
        _CACHE["nc"] = _build()
    nc = _CACHE["nc"]

    in_maps = _make_in_maps(x, w_q, b_q, w_k, b_k, w_v, b_v, w_o, b_o)

    res = run_bass_kernel_spmd(nc, in_maps, core_ids=list(range(NCORES)),
                               trace=_TRACE)
    _LAST = res

    out = np.empty((B, T, C), dtype=np.float32)
    for c in range(NCORES):
        b, g = c // GRP, c % GRP
        o = np.asarray(res.results[c]["outRS"], dtype=np.float32)
        for ts_ in range(NQ):
            out[b, ts_ * TS:(ts_ + 1) * TS, g * JL:(g + 1) * JL] = o[ts_].T
    return out


# revision 24
# speedup vs baseline: 1.0318x; 1.0318x over previous
"""Multi-head causal self-attention (B=2, T=2048, D=1024, H=16) on 8 trn2 cores.

Sharding: core c handles batch b=c//4 and head-group g=c%4 (4 heads, 256 feats).
The kernel streams token supertiles of 512: for each ts it computes Q/K/V for
the core's 4 heads, runs causal attention for query block qs=ts (K/V for all
needed kv blocks are already resident -- causal), multiplies the local 256
attention features into a partial output [1024, 512] against w_o rows, and
ReduceScatters that partial (bf16, op=add) across the 4 cores of the batch
group -- each core receives its own 256 output features for those 512 tokens.
The 4 chunked ReduceScatters overlap the next supertile's compute, replacing
the two serialized full-width AllGathers of the tensor-parallel formulation.

All matmuls run in fp32r (full-rate PE). Softmax skips the running max
(scores bounded ~N(0,1) by construction); the 1/sqrt(Dh) scale is folded into
the Exp activation; the denominator comes from a ones-column appended to V.
Scores are computed as S^T[k, q] so softmax reduces along the free axis and
P@V consumes exp(S^T) directly -- no transposes. The two heads of an f-tile
sit at partition bases 0/64 so their K=64 score matmuls occupy disjoint PE
row-groups; their exps fuse into one 1024-column ACT op. S of block kb+1 is
issued before PV of block kb so the PE never waits on the Exp. Warm-up
matmuls on the resident mask tile ramp the PE clock under the input-DMA
lead-in. b_o/4 is added to each partial pre-reduce, so the RS output is final.
"""

import os
import sys

for _p in ("/opt/trn_rl_repo", "/root/.axon_site/_ro/trn_rl_repo"):
    if os.path.isdir(_p) and _p not in sys.path:
        sys.path.insert(0, _p)

import numpy as np

import concourse.bacc as bacc
import concourse.mybir as mybir
import concourse.tile as tile
from concourse.bass_utils import run_bass_kernel_spmd

F32 = mybir.dt.float32
F32R = mybir.dt.float32r
BF16 = mybir.dt.bfloat16
AF = mybir.ActivationFunctionType

B, T, C = 2, 2048, 1024
H, Dh = 16, 64
NCORES, GRP = 8, 4        # 2 groups of 4 cores (one per batch)
HL, FL = 4, 256           # heads / features per core
TS = 512                  # token supertile
NQ = T // TS              # 4
JL = C // GRP             # 256 output features per core after the RS

_CACHE = {}
_TRACE = False
_LAST = None


def _build(unroll=1):
    nc = bacc.Bacc("TRN2", target_bir_lowering=False, debug=False,
                   num_devices=NCORES)

    xT = nc.dram_tensor("xT", [C, T], F32R, kind="ExternalInput")
    wqT = nc.dram_tensor("wqT", [C, FL], F32R, kind="ExternalInput")
    wkT = nc.dram_tensor("wkT", [C, FL], F32R, kind="ExternalInput")
    wvT = nc.dram_tensor("wvT", [C, FL], F32R, kind="ExternalInput")
    woL = nc.dram_tensor("woL", [FL, C], F32R, kind="ExternalInput")
    bqk_row = nc.dram_tensor("bqk_row", [1, 4, 128], F32R, kind="ExternalInput")
    ones_in = nc.dram_tensor("ones_in", [128, 64], BF16, kind="ExternalInput")
    ones_rin = nc.dram_tensor("ones_rin", [1, TS], F32R, kind="ExternalInput")
    bv_row = nc.dram_tensor("bv_row", [1, FL], F32R, kind="ExternalInput")
    bo_bc = nc.dram_tensor("bo_bc", [128, 8], F32, kind="ExternalInput")
    mask2 = nc.dram_tensor("mask2", [128, 2, 128], BF16, kind="ExternalInput")
    outRS = nc.dram_tensor("outRS", [NQ, JL, TS], BF16, kind="ExternalOutput")

    with tile.TileContext(nc) as tc:
        for _it in range(unroll):
            with tc.tile_pool(name="persist", bufs=1) as pp:
                # ---- persistent SBUF state ----
                QT = pp.tile([128, 2, T], F32R)          # Q^T  [f, t]
                KT = pp.tile([128, 2, T], F32R)          # K^T  [f, t]
                Vg = pp.tile([128, T // 128, HL, Dh + 1], BF16)
                attnT = pp.tile([128, 2, T], F32R)       # attention out^T
                mask_sb = pp.tile([128, 2, 128], BF16)
                bqkr_sb = pp.tile([1, 4, 128], F32R)
                bvr_sb = pp.tile([1, FL], F32R)
                bo_sb = pp.tile([128, 8], F32)
                ones_sb = pp.tile([128, 64], BF16)
                ones_row = pp.tile([1, TS], F32R)

                nc.sync.dma_start(mask_sb[:], mask2[:])
                # PE warm-up ramps the HAM clock under the input-DMA lead-in;
                # psum is discarded.
                with tc.tile_pool(name="warm", bufs=1, space="PSUM") as wp:
                    ps_w = wp.tile([128, 256], F32, name="ps_w")
                    for _w in range(62):
                        nc.tensor.matmul(
                            ps_w[:], lhsT=mask_sb[:, 0, :],
                            rhs=mask_sb.rearrange("p a b -> p (a b)"),
                            start=True, stop=True)

                dp = tc.tile_pool(name="dram", bufs=1, space="DRAM")
                dpp = dp.__enter__()
                rs_in = dpp.tile([NQ, C, TS], BF16)
                rs_out = dpp.tile([NQ, JL, TS], BF16)

                with tc.tile_pool(name="xw", bufs=1) as xw, \
                     tc.tile_pool(name="att", bufs=3) as att, \
                     tc.tile_pool(name="fin2", bufs=2) as fin2, \
                     tc.tile_pool(name="psA", bufs=2, space="PSUM") as psA, \
                     tc.tile_pool(name="psS", bufs=2, space="PSUM") as psS, \
                     tc.tile_pool(name="psO", bufs=2, space="PSUM") as psO:
                    xT_sb = xw.tile([128, 8, T], F32R)
                    wq_sb = xw.tile([128, 8, FL], F32R)
                    wk_sb = xw.tile([128, 8, FL], F32R)
                    wv_sb = xw.tile([128, 8, FL], F32R)
                    wo_sb = xw.tile([128, 2, C], F32R)
                    # coalesced loads, one strided DMA each (HWDGE is paced
                    # at ~625 ns per DMACopy), ordered by first use
                    xTr = xT.rearrange("(c p) t -> p c t", p=128)
                    nc.sync.dma_start(xT_sb[:, 0:4, 0:TS], xTr[:, 0:4, 0:TS])
                    nc.sync.dma_start(xT_sb[:, 4:8, 0:TS], xTr[:, 4:8, 0:TS])
                    nc.sync.dma_start(
                        wq_sb[:], wqT.rearrange("(c p) f -> p c f", p=128))
                    nc.sync.dma_start(bqkr_sb[:], bqk_row[:])
                    nc.sync.dma_start(ones_row[:], ones_rin[:])
                    nc.sync.dma_start(bvr_sb[:], bv_row[:])
                    nc.sync.dma_start(ones_sb[:], ones_in[:])
                    # softmax denominator column of V via a strided ACT copy
                    nc.scalar.activation(
                        Vg[:, :, :, Dh:Dh + 1],
                        ones_sb.rearrange("p (a b o) -> p a b o", a=T // 128,
                                          b=HL), AF.Copy)
                    nc.sync.dma_start(
                        wk_sb[:], wkT.rearrange("(c p) f -> p c f", p=128))
                    nc.sync.dma_start(
                        wv_sb[:], wvT.rearrange("(c p) f -> p c f", p=128))
                    nc.sync.dma_start(xT_sb[:, :, TS:2 * TS], xTr[:, :, TS:2 * TS])
                    nc.sync.dma_start(
                        wo_sb[:], woL.rearrange("(c p) j -> p c j", p=128))
                    nc.sync.dma_start(bo_sb[:], bo_bc[:])
                    for ts_ in range(2, NQ):
                        nc.sync.dma_start(
                            xT_sb[:, :, ts_ * TS:(ts_ + 1) * TS],
                            xTr[:, :, ts_ * TS:(ts_ + 1) * TS])

                    def qkv_gen(ts_):
                        # Q^T,K^T [f, t] for both f-tiles of this supertile.
                        # Biases ride as an extra rank-1 accumulate matmul so
                        # the psum drain is a pure ACT copy -- DVE stays free
                        # for the softmax-normalize chain.
                        for dst, w_sb, bcol in ((QT, wq_sb, 0), (KT, wk_sb, 2)):
                            for ft in range(2):
                                ps = psA.tile([128, TS], F32, name="ps_qk",
                                              tag="psA")
                                nc.tensor.matmul(
                                    ps[:], lhsT=bqkr_sb[0:1, bcol + ft, :],
                                    rhs=ones_row[:], start=True, stop=False)
                                for cc in range(8):
                                    nc.tensor.matmul(
                                        ps[:],
                                        lhsT=w_sb[:, cc, ft * 128:(ft + 1) * 128],
                                        rhs=xT_sb[:, cc, ts_ * TS:(ts_ + 1) * TS],
                                        start=False, stop=(cc == 7))
                                nc.scalar.activation(
                                    dst[:, ft, ts_ * TS:(ts_ + 1) * TS], ps[:],
                                    AF.Copy)
                                yield
                        # V token-major: [t, f] = sum_c x^T[c, t] w_v^T[c, f]
                        for tb in range(4 * ts_, 4 * ts_ + 4):
                            ps = psA.tile([128, TS], F32, name="ps_v",
                                          tag="psA")[:, :FL]
                            nc.tensor.matmul(
                                ps[:], lhsT=ones_row[0:1, 0:128],
                                rhs=bvr_sb[:], start=True, stop=False)
                            for cc in range(8):
                                nc.tensor.matmul(
                                    ps[:],
                                    lhsT=xT_sb[:, cc, tb * 128:(tb + 1) * 128],
                                    rhs=wv_sb[:, cc, :],
                                    start=False, stop=(cc == 7))
                            nc.scalar.activation(
                                Vg[:, tb, :, 0:Dh],
                                ps.rearrange("p (h d) -> p h d", h=HL),
                                AF.Copy)
                            yield

                    def att_gen(qs):
                        # all 4 heads for query supertile qs; heads (2ft, 2ft+1)
                        # at partition bases (0, 64)
                        for ft in range(2):
                            Q0, K0 = QT[0:64, ft, :], KT[0:64, ft, :]
                            Q1, K1 = QT[64:128, ft, :], KT[64:128, ft, :]
                            h0, h1 = 2 * ft, 2 * ft + 1
                            po0 = psO.tile([128, TS], F32, name="po0",
                                           tag="ps_o")
                            po1 = psO.tile([128, TS], F32, name="po1",
                                           tag="ps_o")
                            nkb = 4 * qs + 4

                            def s_part(kb, q_lo):
                                # S^T for both heads at kv block kb -> exp
                                ps_s = psS.tile([128, 2, TS], F32,
                                                name="ps_s", tag="ps_s")
                                nc.tensor.matmul(
                                    ps_s[:, 0, q_lo:TS],
                                    lhsT=K0[:, kb * 128:(kb + 1) * 128],
                                    rhs=Q0[:, qs * TS + q_lo:(qs + 1) * TS],
                                    start=True, stop=True)
                                nc.tensor.matmul(
                                    ps_s[:, 1, q_lo:TS],
                                    lhsT=K1[:, kb * 128:(kb + 1) * 128],
                                    rhs=Q1[:, qs * TS + q_lo:(qs + 1) * TS],
                                    start=True, stop=True)
                                p_sb = att.tile([128, 2, TS], BF16,
                                                name="p_sb", tag="p")
                                nc.scalar.activation(
                                    p_sb[:, :, q_lo:TS], ps_s[:, :, q_lo:TS],
                                    AF.Exp, scale=0.125)
                                diag = kb - 4 * qs
                                if diag >= 0:  # triangular mask
                                    mo = diag * 128
                                    nc.vector.tensor_mul(
                                        p_sb[:, :, mo:mo + 128],
                                        p_sb[:, :, mo:mo + 128],
                                        mask_sb[:])
                                return p_sb

                            def pv_part(kb, q_lo, p_sb):
                                nc.tensor.matmul(
                                    po0[0:65, q_lo:TS],
                                    lhsT=Vg[:, kb, h0, :],
                                    rhs=p_sb[:, 0, q_lo:TS],
                                    start=(kb == 0), stop=(kb == nkb - 1))
                                nc.tensor.matmul(
                                    po1[0:65, q_lo:TS],
                                    lhsT=Vg[:, kb, h1, :],
                                    rhs=p_sb[:, 1, q_lo:TS],
                                    start=(kb == 0), stop=(kb == nkb - 1))

                            # one-block software pipeline: S(kb+1) issues
                            # before PV(kb) so PE never waits on the Exp
                            qlo = lambda kb: max(0, (kb - 4 * qs) * 128)
                            prev = None
                            for kb in range(nkb):
                                cur = (kb, qlo(kb), s_part(kb, qlo(kb)))
                                if prev is not None:
                                    pv_part(*prev)
                                prev = cur
                                yield
                            pv_part(*prev)

                            # normalize both heads by the ones-column sums
                            for po, fb in ((po0, 0), (po1, 64)):
                                r_sb = att.tile([1, TS], F32R, name="r_sb",
                                                tag="r")
                                with nc.allow_low_precision(reason="f32r"):
                                    nc.vector.reciprocal(r_sb[:],
                                                         po[64:65, :])
                                r_bc = att.tile([64, TS], F32R, name="r_bc",
                                                tag="r_bc", bufs=2)
                                nc.gpsimd.partition_broadcast(
                                    r_bc[:], r_sb[:])
                                nc.vector.tensor_mul(
                                    attnT[fb:fb + 64, ft,
                                          qs * TS:(qs + 1) * TS],
                                    po[0:64, :], r_bc[:])
                            yield

                    def wo_gen(ts_, deep=False):
                        # partial out^T[j, t] = w_o[j, f_local] attnT[f_local, t]
                        # (+ b_o/4) -> bf16 -> RS input; epilogues alternate
                        # ACT/DVE so neither engine paces the psum recycling
                        o_sb = fin2.tile([128, 8, TS], BF16, name="o_sb",
                                         tag="o")
                        for jt in range(8):
                            if deep and jt % 2:
                                ps = psO.tile([128, TS], F32, name="po0",
                                              tag="ps_o")
                            else:
                                ps = psA.tile([128, TS], F32, name="ps_f",
                                              tag="psA")
                            for fc in range(2):
                                nc.tensor.matmul(
                                    ps[:],
                                    lhsT=wo_sb[:, fc, jt * 128:(jt + 1) * 128],
                                    rhs=attnT[:, fc, ts_ * TS:(ts_ + 1) * TS],
                                    start=(fc == 0), stop=(fc == 1))
                            # half-width epilogues on ACT and DVE in parallel
                            # keep the psum recycle faster than the matmuls
                            with nc.allow_low_precision(reason="bf16 partial"):
                                nc.scalar.activation(
                                    o_sb[:, jt, 0:TS // 2], ps[:, 0:TS // 2],
                                    AF.Identity, bias=bo_sb[:, jt:jt + 1])
                                nc.vector.tensor_scalar_add(
                                    o_sb[:, jt, TS // 2:TS],
                                    ps[:, TS // 2:TS], bo_sb[:, jt:jt + 1])
                            yield
                        rsv = rs_in[ts_].rearrange("(j p) t -> p j t", p=128)
                        nc.sync.dma_start(rsv[:, 0:4, :], o_sb[:, 0:4, :])
                        nc.sync.dma_start(rsv[:, 4:8, :], o_sb[:, 4:8, :])
                        nc.gpsimd.collective_compute(
                            "ReduceScatter", mybir.AluOpType.add,
                            replica_groups=[[0, 1, 2, 3], [4, 5, 6, 7]],
                            ins=[rs_in[ts_].opt()], outs=[rs_out[ts_].opt()])

                    def drain(g):
                        for _ in g:
                            pass

                    def weave(main, n_main, fillers, n_fill):
                        # spread n_fill filler steps evenly across the n_main
                        # steps of the ACT-bound attention so its exp chain
                        # hides under the fillers' PE-only matmul groups
                        fi = 0
                        credit = 0.0
                        for _ in main:
                            credit += n_fill / n_main
                            while credit >= 1.0 and fi < len(fillers):
                                try:
                                    next(fillers[fi])
                                    credit -= 1.0
                                except StopIteration:
                                    fi += 1
                        for g in fillers[fi:]:
                            drain(g)

                    drain(qkv_gen(0))
                    weave(att_gen(0), 10, [qkv_gen(1)], 12)
                    weave(att_gen(1), 18, [wo_gen(0), qkv_gen(2)], 20)
                    weave(att_gen(2), 26, [wo_gen(1), qkv_gen(3)], 20)
                    weave(att_gen(3), 34, [wo_gen(2)], 8)
                    drain(wo_gen(3, deep=True))
                    # output copies last: a copy's wait on its RS would block
                    # SP.SEQ and delay later rs_in stages if emitted inline
                    for ts_ in range(NQ):
                        nc.sync.dma_start(outRS[ts_], rs_out[ts_])
                dp.__exit__(None, None, None)

    nc.compile()
    return nc


def _bf(a):
    import ml_dtypes
    return np.asarray(a, dtype=ml_dtypes.bfloat16)


def _make_in_maps(x, w_q, b_q, w_k, b_k, w_v, b_v, w_o, b_o):
    x = np.asarray(x, dtype=np.float32)
    w_q = np.asarray(w_q, dtype=np.float32)
    w_k = np.asarray(w_k, dtype=np.float32)
    w_v = np.asarray(w_v, dtype=np.float32)
    w_o = np.asarray(w_o, dtype=np.float32)
    b_q = np.asarray(b_q, dtype=np.float32)
    b_k = np.asarray(b_k, dtype=np.float32)
    b_v = np.asarray(b_v, dtype=np.float32)
    b_o = np.asarray(b_o, dtype=np.float32)

    mask_t = np.triu(np.ones((128, 128), dtype=np.float32))
    xTs = [np.ascontiguousarray(x[b].T) for b in range(B)]
    bo_t = np.ascontiguousarray((b_o / GRP).reshape(8, 128).T)

    in_maps = []
    for c in range(NCORES):
        b, g = c // GRP, c % GRP
        fsl = slice(g * FL, (g + 1) * FL)
        bqk_r = np.concatenate([b_q[fsl].reshape(2, 128),
                                b_k[fsl].reshape(2, 128)])[None]  # [1, 4, 128]
        in_maps.append({
            "xT": xTs[b],
            "wqT": np.ascontiguousarray(w_q[fsl, :].T),
            "wkT": np.ascontiguousarray(w_k[fsl, :].T),
            "wvT": np.ascontiguousarray(w_v[fsl, :].T),
            "woL": np.ascontiguousarray(w_o[:, fsl].T),
            "bqk_row": np.ascontiguousarray(bqk_r),
            "bv_row": np.ascontiguousarray(b_v[fsl][None]),
            "bo_bc": bo_t,
            "mask2": _bf(np.ascontiguousarray(
                np.repeat(mask_t[:, None, :], 2, axis=1))),
            "ones_in": _bf(np.ones((128, 64), dtype=np.float32)),
            "ones_rin": np.ones((1, TS), dtype=np.float32),
        })
    return in_maps


def kernel(x, w_q, b_q, w_k, b_k, w_v, b_v, w_o, b_o):
    global _LAST
    if "nc" not in _CACHE:
        _CACHE["nc"] = _build()
    nc = _CACHE["nc"]

    in_maps = _make_in_maps(x, w_q, b_q, w_k, b_k, w_v, b_v, w_o, b_o)

    res = run_bass_kernel_spmd(nc, in_maps, core_ids=list(range(NCORES)),
                               trace=_TRACE)
    _LAST = res

    out = np.empty((B, T, C), dtype=np.float32)
    for c in range(NCORES):
        b, g = c // GRP, c % GRP
        o = np.asarray(res.results[c]["outRS"], dtype=np.float32)
        for ts_ in range(NQ):
            out[b, ts_ * TS:(ts_ + 1) * TS, g * JL:(g + 1) * JL] = o[ts_].T
    return out
